# revision 6
# baseline (speedup 1.0000x reference)
"""CTC loss kernel for Trainium2 (8 NeuronCores, data-parallel over batch).

Strategy
--------
Per core: 64 examples. The CTC forward DP runs in probability space with
states in the free dim and (example, direction) packed into the 128
partitions: rows 0-63 run the forward DP for t=0..255, rows 64-127 run
the suffix (backward) DP in state-reversed coordinates for t=511..256.
The two halves are spliced at T/2 on host: P = sum_s alpha_255[s]*W_255[s].

Emissions are produced on the TensorEngine: per (example, 128-t block),
PE-transpose y_pred (pre-cast to bf16 on GpSimd) to (class, t), then a
widened one-hot matmul gathers BOTH the skip-masked and plain emissions
for 128 t steps in one N=264 instruction:
  EC[t, 0:132]   = m2[s]*G*y_pred[t, ext[s]]   (skip path, mask folded in)
  EC[t, 132:264] =       G*y_pred[t, ext[s]]   (stay/step path)
A DRAM round-trip reshuffles (t-part, ex, 264) into the DP's
(example-part, tau-major) chunk layout.

The DP step is 3 DVE instructions using a packed state tile
T = [alpha(132) | guard(2) | u(132)]:
  1. u = alpha[0:132] + alpha[1:133]              (T[134:266])
  2. W = T[2:266] * EC_tau                        ([a2*ecm | u*ec], 2x mode)
  3. alpha = W[0:132] + W[132:264]                (2x mode)
States are MIRRORED (guards at top) so ops 2-3 have 4-byte-aligned
operands and hit the DVE 2x bf16 perf mode.

Numerics: bf16 state, per-16-step rescale to a 2^43 setpoint via the
fast approx reciprocal (max history written out, logs added on host).
Masked/pad one-hot columns are exactly zero (no eps bias; the eps in
log(y+eps) shifts the loss by ~1e-5 relative, far under tolerance).
The final splice spans ~e^-180 for tail examples, so the two final
state tiles are DMA'd out and spliced on host in f64.
"""

import numpy as np

B, T, C, L = 512, 512, 96, 64
BLANK = C - 1
S = 2 * L + 1          # 129 states
SW = 132               # padded state width (multiple of 4)
EW = 2 * SW            # emission width per (example, t): [ecm | ec]
NCORES = 8
BN = B // NCORES       # 64 examples per core
TH = T // 2            # split point
RESC = 16              # rescale period
NRESC = (TH - 1) // RESC  # 15 rescales
SETPOINT_LOG2 = 43     # rescale normalizes row max to 2^43
G = 60.646622          # exp(mean_loss/T) boost; keeps alpha ~O(1) per step

_BUILT = None
_LAST_EXEC_NS = None
_LAST_RES = None


def _host_metadata(y_true):
    """ext labels, skip masks, init masks, per-state classes — from y_true."""
    y_true = np.asarray(y_true, dtype=np.int32)
    lbl_len = (y_true != -1).sum(axis=-1).astype(np.int32)
    labels = np.where(y_true != -1, y_true, 0).astype(np.int32)
    ext = np.full((B, S), BLANK, np.int32)
    ext[:, 1::2] = labels
    ext_m2 = np.pad(ext[:, :-2], ((0, 0), (2, 0)), constant_values=BLANK)
    can_skip = ((ext != BLANK) & (ext != ext_m2)).astype(np.float32)

    m2f = np.zeros((B, SW), np.float32)
    m2f[:, :S] = can_skip
    m2b = np.zeros((B, SW), np.float32)
    for u in range(2, S):
        m2b[:, u] = can_skip[:, S - 1 - u + 2]

    mif = np.zeros((B, SW), np.float32)
    mif[:, 0] = 1.0
    mif[:, 1] = 1.0
    mib = np.zeros((B, SW), np.float32)
    mib[np.arange(B), S - 1 - 2 * lbl_len] = 1.0
    mib[np.arange(B), S - 1 - (2 * lbl_len - 1)] = 1.0

    clsf = np.full((B, SW), -1, np.int32)           # -1 -> all-zero column
    clsf[:, :S] = ext
    clsb = np.full((B, SW), -1, np.int32)
    clsb[:, :S] = ext[:, ::-1]
    return m2f, m2b, mif, mib, clsf, clsb


def _build(num_cores=NCORES, t_full=T, bn=BN):
    """Build and schedule the Bass module once."""
    import concourse.bacc as bacc
    import concourse.mybir as mybir
    import concourse.tile as tile
    from contextlib import ExitStack
    from concourse.vector_clock import ScopedClock

    # split the TileContext end-drain's waits across a chain of drains
    # (single sem wait per drain instruction).
    def _patched_drain_and_barrier(self, tick_clock, wait_clock):
        nc = self.nc
        drain_inst = nc.sync.drain()
        wait_clock.add_sem_waits(
            drain_inst.ins, ScopedClock({None: tick_clock.global_clock})
        )
        si = drain_inst.ins.sync_info
        waits = list(si.on_wait) if si and si.on_wait else []
        if len(waits) > 1:
            si.on_wait = waits[:1]
            for w in waits[1:]:
                extra = nc.sync.drain()
                esi = extra.ins.sync_info
                if esi is None:
                    extra.ins.sync_info = mybir.SyncInfo(on_wait=[w], on_update=[])
                else:
                    esi.on_wait = (esi.on_wait or []) + [w]
        nc.all_engine_barrier()
        assert self.sems is not None
        popped = nc._tile_sem_poison_stack.pop()
        assert popped is self._sem_poison
        nc.clear_and_free_semaphores(list(self.sems.allocated().values()))
        nc.all_engine_barrier()

    tile.TileContext._drain_and_barrier = _patched_drain_and_barrier

    f32 = mybir.dt.float32
    bf16 = mybir.dt.bfloat16
    AX = mybir.AxisListType.X
    COPY = mybir.ActivationFunctionType.Copy
    MULT = mybir.AluOpType.mult

    th = t_full // 2
    nblk = t_full // 128
    chk = 32
    nchk = th // chk
    nresc = (th - 1) // RESC

    nc = bacc.Bacc("TRN2", target_bir_lowering=False, debug=False,
                   num_devices=num_cores)
    ypred = nc.dram_tensor("ypred", [bn, t_full, C], f32, kind="ExternalInput")
    m2_in = nc.dram_tensor("m2", [128, SW], bf16, kind="ExternalInput")
    mi_in = nc.dram_tensor("minit", [128, SW], bf16, kind="ExternalInput")
    oh_in = nc.dram_tensor("onehot", [C, bn * 2 * EW], bf16, kind="ExternalInput")
    id_in = nc.dram_tensor("ident", [128, 128], bf16, kind="ExternalInput")
    h_out = nc.dram_tensor("hist", [128, max(nresc, 1)], f32, kind="ExternalOutput")
    a_out = nc.dram_tensor("afin", [128, SW + 2], bf16, kind="ExternalOutput")
    g_out = nc.dram_tensor("gfin", [128, SW], bf16, kind="ExternalOutput")

    with tile.TileContext(nc) as tc, ExitStack() as ctx:
        const = ctx.enter_context(tc.tile_pool(name="const", bufs=1))
        dramp = ctx.enter_context(tc.tile_pool(name="edram", bufs=1, space="DRAM"))
        ypf_pool = ctx.enter_context(tc.tile_pool(name="ypf", bufs=2))
        ypb_pool = ctx.enter_context(tc.tile_pool(name="ypb", bufs=2))
        stg_pool = ctx.enter_context(tc.tile_pool(name="stg", bufs=4))
        ec_pool = ctx.enter_context(tc.tile_pool(name="ec", bufs=2))
        ytp_pool = ctx.enter_context(tc.tile_pool(name="ytp", bufs=2, space="PSUM"))
        eps_pool = ctx.enter_context(tc.tile_pool(name="eps", bufs=4, space="PSUM"))
        yts_pool = ctx.enter_context(tc.tile_pool(name="yts", bufs=2))

        m2t = const.tile([128, SW], bf16, tag="m2t")
        mit = const.tile([128, SW], bf16, tag="mit")
        oht = const.tile([C, bn * 2 * EW], bf16, tag="oht")
        identt = const.tile([128, 128], bf16, tag="identt")
        Tt = const.tile([128, 2 * SW + 2], bf16, tag="Tt")   # [alpha|guard|u]
        Wt = const.tile([128, EW], bf16, tag="Wt")
        ut = const.tile([128, SW], bf16, tag="ut")
        vt = const.tile([128, SW], bf16, tag="vt")
        wt = const.tile([128, SW], bf16, tag="wt")
        histt = const.tile([128, max(nresc, 1)], f32, tag="histt")
        sclt = const.tile([128, 1], f32, tag="sclt")

        nc.sync.dma_start(out=m2t[:, :], in_=m2_in.ap())
        nc.sync.dma_start(out=mit[:, :], in_=mi_in.ap())
        nc.sync.dma_start(out=oht[:, :], in_=oh_in.ap())
        nc.sync.dma_start(out=identt[:, :], in_=id_in.ap())
        nc.vector.memset(histt[:, :], 0.0)
        nc.vector.memset(Tt[:, :], 0.0)

        # ---- phase A: emissions per 128-t block via PE one-hot matmul ----
        blk_order = []
        for i in range(nblk // 2):
            blk_order += [i, nblk - 1 - i]
        edram = {}
        ec_tiles = {}
        for k in blk_order:
            dirn = 0 if k < nblk // 2 else 1
            ed = dramp.tile([128, bn * EW], bf16, tag=f"ed{k}")
            edram[k] = ed
            for grp in range(bn // 16):
                ypf = ypf_pool.tile([128, 16 * C], f32)
                yp3 = ypf[:, :].rearrange("p (e c) -> p e c", c=C)
                src = ypred.ap()[grp * 16:(grp + 1) * 16,
                                 k * 128:(k + 1) * 128, :]
                nc.sync.dma_start(out=yp3[:, :, :],
                                  in_=src.rearrange("e t c -> t e c"))
                ypb = ypb_pool.tile([128, 16 * C], bf16)
                nc.gpsimd.tensor_copy(ypb[:, :], ypf[:, :])
                for q in range(4):          # 4 examples per PSUM group
                    ytp = ytp_pool.tile([C, 512], bf16)     # half a bank
                    for e4 in range(4):
                        e = q * 4 + e4
                        nc.tensor.transpose(
                            ytp[:, e4 * 128:(e4 + 1) * 128],
                            ypb[:, e * C:(e + 1) * C],
                            identt[:, :])
                    yts = yts_pool.tile([C, 512], bf16)
                    nc.scalar.activation(yts[:, :], ytp[:, :], COPY,
                                         bias=0.0, scale=1.0)
                    stg = stg_pool.tile([128, 4 * EW], bf16)
                    for e4 in range(4):
                        e = q * 4 + e4
                        ex = grp * 16 + e
                        ohoff = (ex * 2 + dirn) * EW
                        epsum = eps_pool.tile([128, EW], f32)
                        nc.tensor.matmul(
                            epsum[:, :],
                            yts[:, e4 * 128:(e4 + 1) * 128],
                            oht[:, ohoff:ohoff + EW],
                            start=True, stop=True)
                        nc.scalar.activation(
                            stg[:, e4 * EW:(e4 + 1) * EW], epsum[:, :],
                            COPY, bias=0.0, scale=float(G))
                    ex0 = grp * 16 + q * 4
                    nc.gpsimd.dma_start(
                        out=ed[:, ex0 * EW:(ex0 + 4) * EW], in_=stg[:, :])

            # ---- phase B: chunks for this block pair (after 2nd block) ----
            if k >= nblk // 2:
                kf = nblk - 1 - k
                kb = k
                for jj in range(128 // chk):
                    j = kf * (128 // chk) + jj
                    ec = ec_pool.tile([128, chk * EW], bf16)
                    ec3 = ec[:, :].rearrange("p (t s) -> p t s", s=EW)
                    tl0 = jj * chk
                    tb0 = 127 - jj * chk
                    fsrc = edram[kf][:, :].rearrange("t (e s) -> t e s", s=EW)
                    bsrc = edram[kb][:, :].rearrange("t (e s) -> t e s", s=EW)
                    bslice = (slice(tb0, None, -1) if tb0 - chk < 0
                              else slice(tb0, tb0 - chk, -1))
                    nc.sync.dma_start(
                        out=ec3[0:64, :, :],
                        in_=fsrc[tl0:tl0 + chk, :, :].rearrange("t e s -> e t s"))
                    nc.gpsimd.dma_start(
                        out=ec3[64:128, :, :],
                        in_=bsrc[bslice, :, :].rearrange("t e s -> e t s"))
                    ec_tiles[j] = ec

        # ---- phase C: the DP (mirrored states, 3 DVE ops per step) ----
        nc.vector.tensor_mul(Tt[:, 0:SW], ec_tiles[0][:, SW:EW], mit[:, :])
        nr = 0
        for tau in range(1, th):
            ec = ec_tiles[tau // chk]
            off = (tau % chk) * EW
            nc.vector.tensor_add(Tt[:, SW + 2:2 * SW + 2],
                                 Tt[:, 0:SW], Tt[:, 1:1 + SW])
            nc.vector.tensor_mul(Wt[:, :], Tt[:, 2:2 * SW + 2],
                                 ec[:, off:off + EW])
            nc.vector.tensor_add(Tt[:, 0:SW], Wt[:, 0:SW], Wt[:, SW:EW])
            if tau % RESC == 0 and nr < nresc:
                nc.vector.reduce_max(histt[:, nr:nr + 1], Tt[:, 2:SW], axis=AX)
                nc.vector.reciprocal_approx_fast(sclt[:, :], histt[:, nr:nr + 1])
                nc.vector.tensor_scalar(Tt[:, 0:SW], Tt[:, 0:SW],
                                        sclt[:, :], float(2.0 ** SETPOINT_LOG2),
                                        MULT, MULT)
                nr += 1

        # ---- final: gamma on bwd rows, dump states (host f64 splice) ----
        nc.vector.tensor_add(ut[:, :], Tt[:, 0:SW], Tt[:, 1:1 + SW])
        nc.vector.tensor_mul(vt[:, :], Tt[:, 2:2 + SW], m2t[:, :])
        nc.vector.tensor_add(wt[:, :], ut[:, :], vt[:, :])
        nc.sync.dma_start(out=a_out.ap(), in_=Tt[:, 0:SW + 2])
        nc.sync.dma_start(out=g_out.ap(), in_=wt[:, :])
        nc.sync.dma_start(out=h_out.ap(), in_=histt[:, :])

    nc.compile()
    return nc


def kernel(y_true, y_pred):
    global _BUILT, _LAST_EXEC_NS, _LAST_RES
    from concourse.bass_utils import run_bass_kernel_spmd

    y_true = np.asarray(y_true)
    y_pred = np.ascontiguousarray(np.asarray(y_pred, dtype=np.float32))

    m2f, m2b, mif, mib, clsf, clsb = _host_metadata(y_true)

    if _BUILT is None:
        _BUILT = _build()
    nc = _BUILT

    import ml_dtypes
    bf = ml_dtypes.bfloat16
    ident = np.eye(128, dtype=np.float32)
    in_maps = []
    for c in range(NCORES):
        sl = slice(c * BN, (c + 1) * BN)
        # mirrored layout: reverse the free (state) dim
        m2 = np.concatenate([m2f[sl], m2b[sl]], axis=0)[:, ::-1].astype(bf)
        mi = np.concatenate([mif[sl], mib[sl]], axis=0)[:, ::-1].astype(bf)
        # widened one-hot: per (ex, dir) EW=264 cols: [OH*m2 | OH] (mirrored)
        oh = np.zeros((C, BN * 2 * EW), np.float32)
        for e in range(BN):
            b = c * BN + e
            for dirn, cls, m2row in ((0, clsf[b], m2f[b]), (1, clsb[b], m2b[b])):
                colbase = (e * 2 + dirn) * EW
                rcls = cls[::-1]
                rm2 = m2row[::-1]
                valid = rcls >= 0
                idx = np.nonzero(valid)[0]
                oh[rcls[idx], colbase + idx] = rm2[idx]
                oh[rcls[idx], colbase + SW + idx] = 1.0
        in_maps.append({
            "ypred": y_pred[sl],
            "m2": np.ascontiguousarray(m2),
            "minit": np.ascontiguousarray(mi),
            "onehot": oh.astype(bf),
            "ident": ident.astype(bf),
        })

    import os
    trace = os.environ.get("CTC_TRACE", "") == "1"
    res = run_bass_kernel_spmd(nc, in_maps, list(range(NCORES)), trace=trace)
    _LAST_EXEC_NS = res.exec_time_ns
    _LAST_RES = res

    losses = np.zeros(B, np.float64)
    lng = np.log(np.float64(G))
    setlog = NRESC * SETPOINT_LOG2 * np.log(2.0)
    for c in range(NCORES):
        afin = res.results[c]["afin"].astype(np.float64)   # (128, SW+2) mirrored
        gfin = res.results[c]["gfin"].astype(np.float64)   # (128, SW) mirrored
        hist = res.results[c]["hist"].astype(np.float64)
        acc = np.log(np.maximum(hist[:, :NRESC], 1e-300)).sum(axis=1)
        afs = afin[:, 0:SW][:, ::-1]             # un-mirror -> natural order
        gfs = gfin[:, :][:, ::-1]
        af = afs[0:64, 0:S]                      # alpha_{T/2-1}[s]
        gm = gfs[64:128, 0:S][:, ::-1]           # W_{T/2-1}[s], u -> s
        P = (af * gm).sum(axis=1)
        lnP = np.log(np.maximum(P, 1e-300))
        losses[c * BN:(c + 1) * BN] = -(
            lnP + acc[:64] + acc[64:128] - 2 * setlog - T * lng)
    return np.float32(losses.mean())


# revision 7
# speedup vs baseline: 1.0785x; 1.0785x over previous
"""CTC loss kernel for Trainium2 (8 NeuronCores, data-parallel over batch).

Strategy
--------
Per core: 64 examples. The CTC forward DP runs in probability space with
states in the free dim and (example, direction) packed into the 128
partitions: rows 0-63 run the forward DP for t=0..255, rows 64-127 run
the suffix (backward) DP in state-reversed coordinates for t=511..256.
The two halves are spliced at T/2 on host: P = sum_s alpha_255[s]*W_255[s].

Emissions are produced on the TensorEngine: per (example, 128-t block),
PE-transpose y_pred (pre-cast to bf16) to (class, t), then a one-hot
matmul gathers all 132 state emissions for 128 t steps (N=132). A DRAM
round-trip reshuffles (t-part, ex, 132) into the DP's (example-part,
tau-major) chunk layout, landing in the ec half of an interleaved
[ecm(132) | ec(132)] per-tau layout; a bulk DVE multiply by the
replicated skip mask fills the ecm half (ecm = m2*ec).

The DP step is 3 DVE instructions using packed state tiles
T = [alpha(132) | guard(2) | u(132)] (ping-ponged T0/T1 to relax
in-place hazards):
  1. u = alpha[0:132] + alpha[1:133]           (T_cur[134:266])
  2. W = T_cur[2:266] * ECC_tau                ([a2*ecm | u*ec], 2x mode)
  3. alpha' = W[0:132] + W[132:264]            (into T_next[0:132])
States are MIRRORED (guards at top) so ops 2-3 have 4-byte-aligned
operands and hit the DVE 2x bf16 perf mode.

Numerics: bf16 state, per-16-step rescale to a 2^43 setpoint via the
fast approx reciprocal (max history written out, logs added on host).
Masked/pad one-hot columns are exactly zero (no eps bias; the eps in
log(y+eps) shifts the loss by ~1e-5 relative, far under tolerance).
The final splice spans ~e^-180 for tail examples, so the two final
state tiles are DMA'd out and spliced on host in f64.
"""

import numpy as np

B, T, C, L = 512, 512, 96, 64
BLANK = C - 1
S = 2 * L + 1          # 129 states
SW = 132               # padded state width (multiple of 4)
EW = 2 * SW            # per-tau DP emission width: [ecm | ec]
NCORES = 8
BN = B // NCORES       # 64 examples per core
TH = T // 2            # split point
CHK = 32               # tau chunk size
RESC = 16              # rescale period
NRESC = (TH - 1) // RESC  # 15 rescales
SETPOINT_LOG2 = 43     # rescale normalizes row max to 2^43
G = 60.646622          # exp(mean_loss/T) boost; keeps alpha ~O(1) per step

_BUILT = None
_LAST_EXEC_NS = None
_LAST_RES = None


def _host_metadata(y_true):
    """ext labels, skip masks, init masks, per-state classes — from y_true."""
    y_true = np.asarray(y_true, dtype=np.int32)
    lbl_len = (y_true != -1).sum(axis=-1).astype(np.int32)
    labels = np.where(y_true != -1, y_true, 0).astype(np.int32)
    ext = np.full((B, S), BLANK, np.int32)
    ext[:, 1::2] = labels
    ext_m2 = np.pad(ext[:, :-2], ((0, 0), (2, 0)), constant_values=BLANK)
    can_skip = ((ext != BLANK) & (ext != ext_m2)).astype(np.float32)

    m2f = np.zeros((B, SW), np.float32)
    m2f[:, :S] = can_skip
    m2b = np.zeros((B, SW), np.float32)
    for u in range(2, S):
        m2b[:, u] = can_skip[:, S - 1 - u + 2]

    mif = np.zeros((B, SW), np.float32)
    mif[:, 0] = 1.0
    mif[:, 1] = 1.0
    mib = np.zeros((B, SW), np.float32)
    mib[np.arange(B), S - 1 - 2 * lbl_len] = 1.0
    mib[np.arange(B), S - 1 - (2 * lbl_len - 1)] = 1.0

    clsf = np.full((B, SW), -1, np.int32)           # -1 -> all-zero column
    clsf[:, :S] = ext
    clsb = np.full((B, SW), -1, np.int32)
    clsb[:, :S] = ext[:, ::-1]
    return m2f, m2b, mif, mib, clsf, clsb


def _build(num_cores=NCORES, t_full=T, bn=BN):
    """Build and schedule the Bass module once."""
    import concourse.bacc as bacc
    import concourse.mybir as mybir
    import concourse.tile as tile
    from contextlib import ExitStack
    from concourse.vector_clock import ScopedClock

    # split the TileContext end-drain's waits across a chain of drains
    # (single sem wait per drain instruction).
    def _patched_drain_and_barrier(self, tick_clock, wait_clock):
        nc = self.nc
        drain_inst = nc.sync.drain()
        wait_clock.add_sem_waits(
            drain_inst.ins, ScopedClock({None: tick_clock.global_clock})
        )
        si = drain_inst.ins.sync_info
        waits = list(si.on_wait) if si and si.on_wait else []
        if len(waits) > 1:
            si.on_wait = waits[:1]
            for w in waits[1:]:
                extra = nc.sync.drain()
                esi = extra.ins.sync_info
                if esi is None:
                    extra.ins.sync_info = mybir.SyncInfo(on_wait=[w], on_update=[])
                else:
                    esi.on_wait = (esi.on_wait or []) + [w]
        nc.all_engine_barrier()
        assert self.sems is not None
        popped = nc._tile_sem_poison_stack.pop()
        assert popped is self._sem_poison
        nc.clear_and_free_semaphores(list(self.sems.allocated().values()))
        nc.all_engine_barrier()

    tile.TileContext._drain_and_barrier = _patched_drain_and_barrier

    f32 = mybir.dt.float32
    bf16 = mybir.dt.bfloat16
    AX = mybir.AxisListType.X
    COPY = mybir.ActivationFunctionType.Copy
    MULT = mybir.AluOpType.mult

    th = t_full // 2
    nblk = t_full // 128
    chk = CHK
    nresc = (th - 1) // RESC

    nc = bacc.Bacc("TRN2", target_bir_lowering=False, debug=False,
                   num_devices=num_cores)
    ypred = nc.dram_tensor("ypred", [bn, t_full, C], f32, kind="ExternalInput")
    m2_in = nc.dram_tensor("m2", [128, SW], bf16, kind="ExternalInput")
    m2r_in = nc.dram_tensor("m2rep", [128, chk * SW], bf16, kind="ExternalInput")
    mi_in = nc.dram_tensor("minit", [128, SW], bf16, kind="ExternalInput")
    oh_in = nc.dram_tensor("onehot", [C, bn * 2 * SW], bf16, kind="ExternalInput")
    id_in = nc.dram_tensor("ident", [128, 128], bf16, kind="ExternalInput")
    h_out = nc.dram_tensor("hist", [128, max(nresc, 1)], f32, kind="ExternalOutput")
    a_out = nc.dram_tensor("afin", [128, SW + 2], bf16, kind="ExternalOutput")
    g_out = nc.dram_tensor("gfin", [128, SW], bf16, kind="ExternalOutput")

    with tile.TileContext(nc) as tc, ExitStack() as ctx:
        const = ctx.enter_context(tc.tile_pool(name="const", bufs=1))
        dramp = ctx.enter_context(tc.tile_pool(name="edram", bufs=1, space="DRAM"))
        ypf_pool = ctx.enter_context(tc.tile_pool(name="ypf", bufs=2))
        ypb_pool = ctx.enter_context(tc.tile_pool(name="ypb", bufs=2))
        stg_pool = ctx.enter_context(tc.tile_pool(name="stg", bufs=4))
        ec_pool = ctx.enter_context(tc.tile_pool(name="ec", bufs=3))
        ytp_pool = ctx.enter_context(tc.tile_pool(name="ytp", bufs=2, space="PSUM"))
        eps_pool = ctx.enter_context(tc.tile_pool(name="eps", bufs=4, space="PSUM"))
        yts_pool = ctx.enter_context(tc.tile_pool(name="yts", bufs=2))

        m2t = const.tile([128, SW], bf16, tag="m2t")
        m2rep = const.tile([128, chk * SW], bf16, tag="m2rep")
        mit = const.tile([128, SW], bf16, tag="mit")
        oht = const.tile([C, bn * 2 * SW], bf16, tag="oht")
        identt = const.tile([128, 128], bf16, tag="identt")
        T0 = const.tile([128, 2 * SW + 2], bf16, tag="T0")   # [alpha|guard|u]
        T1 = const.tile([128, 2 * SW + 2], bf16, tag="T1")
        W0 = const.tile([128, EW], bf16, tag="W0")
        W1 = const.tile([128, EW], bf16, tag="W1")
        ut = const.tile([128, SW], bf16, tag="ut")
        vt = const.tile([128, SW], bf16, tag="vt")
        wt = const.tile([128, SW], bf16, tag="wt")
        histt = const.tile([128, max(nresc, 1)], f32, tag="histt")
        sclt = const.tile([128, 1], f32, tag="sclt")

        nc.sync.dma_start(out=m2t[:, :], in_=m2_in.ap())
        nc.sync.dma_start(out=m2rep[:, :], in_=m2r_in.ap())
        nc.sync.dma_start(out=mit[:, :], in_=mi_in.ap())
        nc.sync.dma_start(out=identt[:, :], in_=id_in.ap())
        # oht in 16 per-group slices so group 0 can start immediately
        gw = 16 * 2 * SW
        for g in range(bn // 16):
            nc.sync.dma_start(out=oht[:, g * gw:(g + 1) * gw],
                              in_=oh_in.ap()[:, g * gw:(g + 1) * gw])
        nc.vector.memset(histt[:, :], 0.0)
        nc.vector.memset(T0[:, :], 0.0)
        nc.vector.memset(T1[:, :], 0.0)

        # ---- phase A: emissions per 128-t block via PE one-hot matmul ----
        blk_order = []
        for i in range(nblk // 2):
            blk_order += [i, nblk - 1 - i]
        edram = {}
        ecc_tiles = {}
        for kidx, k in enumerate(blk_order):
            dirn = 0 if k < nblk // 2 else 1
            early = kidx < 2            # blocks {0,3}: cast on idle DVE
            ed = dramp.tile([128, bn * SW], bf16, tag=f"ed{k}")
            edram[k] = ed
            for grp in range(bn // 16):
                ypf = ypf_pool.tile([128, 16 * C], f32)
                yp3 = ypf[:, :].rearrange("p (e c) -> p e c", c=C)
                src = ypred.ap()[grp * 16:(grp + 1) * 16,
                                 k * 128:(k + 1) * 128, :]
                nc.sync.dma_start(out=yp3[:, :, :],
                                  in_=src.rearrange("e t c -> t e c"))
                ypb = ypb_pool.tile([128, 16 * C], bf16)
                if early:
                    nc.vector.tensor_copy(ypb[:, :], ypf[:, :])
                else:
                    nc.gpsimd.tensor_copy(ypb[:, :], ypf[:, :])
                for q in range(4):          # 4 examples per PSUM group
                    ytp = ytp_pool.tile([C, 512], bf16)
                    for e4 in range(4):
                        e = q * 4 + e4
                        nc.tensor.transpose(
                            ytp[:, e4 * 128:(e4 + 1) * 128],
                            ypb[:, e * C:(e + 1) * C],
                            identt[:, :])
                    yts = yts_pool.tile([C, 512], bf16)
                    nc.scalar.activation(yts[:, :], ytp[:, :], COPY,
                                         bias=0.0, scale=1.0)
                    stg = stg_pool.tile([128, 4 * SW], bf16)
                    for e4 in range(4):
                        e = q * 4 + e4
                        ex = grp * 16 + e
                        ohoff = (ex * 2 + dirn) * SW
                        epsum = eps_pool.tile([128, SW], f32)
                        nc.tensor.matmul(
                            epsum[:, :],
                            yts[:, e4 * 128:(e4 + 1) * 128],
                            oht[:, ohoff:ohoff + SW],
                            start=True, stop=True)
                        nc.scalar.activation(
                            stg[:, e4 * SW:(e4 + 1) * SW], epsum[:, :],
                            COPY, bias=0.0, scale=float(G))
                    ex0 = grp * 16 + q * 4
                    nc.sync.dma_start(
                        out=ed[:, ex0 * SW:(ex0 + 4) * SW], in_=stg[:, :])

            # ---- phase B: chunks for this block pair (after 2nd block) ----
            if k >= nblk // 2:
                kf = nblk - 1 - k
                kb = k
                for jj in range(128 // chk):
                    j = kf * (128 // chk) + jj
                    ecc = ec_pool.tile([128, chk * EW], bf16)
                    ec3 = ecc[:, :].rearrange("p (t s) -> p t s", s=EW)
                    tl0 = jj * chk
                    tb0 = 127 - jj * chk
                    fsrc = edram[kf][:, :].rearrange("t (e s) -> t e s", s=SW)
                    bsrc = edram[kb][:, :].rearrange("t (e s) -> t e s", s=SW)
                    bslice = (slice(tb0, None, -1) if tb0 - chk < 0
                              else slice(tb0, tb0 - chk, -1))
                    nc.sync.dma_start(
                        out=ec3[0:64, :, SW:EW],
                        in_=fsrc[tl0:tl0 + chk, :, :].rearrange("t e s -> e t s"))
                    nc.sync.dma_start(
                        out=ec3[64:128, :, SW:EW],
                        in_=bsrc[bslice, :, :].rearrange("t e s -> e t s"))
                    # ecm = m2 * ec (bulk, strided write into the ecm half)
                    nc.vector.tensor_mul(
                        ec3[:, :, 0:SW], ec3[:, :, SW:EW],
                        m2rep[:, :].rearrange("p (t s) -> p t s", s=SW))
                    ecc_tiles[j] = ecc

        # ---- phase C: the DP (mirrored states, 3 DVE ops per step) ----
        Ts = (T0, T1)
        Ws = (W0, W1)
        nc.vector.tensor_mul(Ts[0][:, 0:SW],
                             ecc_tiles[0][:, SW:EW], mit[:, :])
        nr = 0
        for tau in range(1, th):
            ecc = ecc_tiles[tau // chk]
            off = (tau % chk) * EW
            Tc = Ts[(tau - 1) % 2]
            Tn = Ts[tau % 2]
            Wc = Ws[tau % 2]
            nc.vector.tensor_add(Tc[:, SW + 2:2 * SW + 2],
                                 Tc[:, 0:SW], Tc[:, 1:1 + SW])
            nc.vector.tensor_mul(Wc[:, :], Tc[:, 2:2 * SW + 2],
                                 ecc[:, off:off + EW])
            nc.vector.tensor_add(Tn[:, 0:SW], Wc[:, 0:SW], Wc[:, SW:EW])
            if tau % RESC == 0 and nr < nresc:
                nc.vector.reduce_max(histt[:, nr:nr + 1], Tn[:, 2:SW], axis=AX)
                nc.vector.reciprocal_approx_fast(sclt[:, :], histt[:, nr:nr + 1])
                nc.vector.tensor_scalar(Tn[:, 0:SW], Tn[:, 0:SW],
                                        sclt[:, :], float(2.0 ** SETPOINT_LOG2),
                                        MULT, MULT)
                nr += 1

        # ---- final: gamma on bwd rows, dump states (host f64 splice) ----
        Tf = Ts[(th - 1) % 2]
        nc.vector.tensor_add(ut[:, :], Tf[:, 0:SW], Tf[:, 1:1 + SW])
        nc.vector.tensor_mul(vt[:, :], Tf[:, 2:2 + SW], m2t[:, :])
        nc.vector.tensor_add(wt[:, :], ut[:, :], vt[:, :])
        nc.sync.dma_start(out=a_out.ap(), in_=Tf[:, 0:SW + 2])
        nc.sync.dma_start(out=g_out.ap(), in_=wt[:, :])
        nc.sync.dma_start(out=h_out.ap(), in_=histt[:, :])

    nc.compile()
    return nc


def kernel(y_true, y_pred):
    global _BUILT, _LAST_EXEC_NS, _LAST_RES
    from concourse.bass_utils import run_bass_kernel_spmd

    y_true = np.asarray(y_true)
    y_pred = np.ascontiguousarray(np.asarray(y_pred, dtype=np.float32))

    m2f, m2b, mif, mib, clsf, clsb = _host_metadata(y_true)

    if _BUILT is None:
        _BUILT = _build()
    nc = _BUILT

    import ml_dtypes
    bf = ml_dtypes.bfloat16
    ident = np.eye(128, dtype=np.float32)
    in_maps = []
    for c in range(NCORES):
        sl = slice(c * BN, (c + 1) * BN)
        # mirrored layout: reverse the free (state) dim
        m2 = np.concatenate([m2f[sl], m2b[sl]], axis=0)[:, ::-1].astype(bf)
        mi = np.concatenate([mif[sl], mib[sl]], axis=0)[:, ::-1].astype(bf)
        m2rep = np.tile(m2, (1, CHK))
        oh = np.zeros((C, BN * 2 * SW), np.float32)
        for e in range(BN):
            b = c * BN + e
            for dirn, cls in ((0, clsf[b]), (1, clsb[b])):
                colbase = (e * 2 + dirn) * SW
                rcls = cls[::-1]
                idx = np.nonzero(rcls >= 0)[0]
                oh[rcls[idx], colbase + idx] = 1.0
        in_maps.append({
            "ypred": y_pred[sl],
            "m2": np.ascontiguousarray(m2),
            "m2rep": np.ascontiguousarray(m2rep),
            "minit": np.ascontiguousarray(mi),
            "onehot": oh.astype(bf),
            "ident": ident.astype(bf),
        })

    import os
    trace = os.environ.get("CTC_TRACE", "") == "1"
    res = run_bass_kernel_spmd(nc, in_maps, list(range(NCORES)), trace=trace)
    _LAST_EXEC_NS = res.exec_time_ns
    _LAST_RES = res

    losses = np.zeros(B, np.float64)
    lng = np.log(np.float64(G))
    setlog = NRESC * SETPOINT_LOG2 * np.log(2.0)
    for c in range(NCORES):
        afin = res.results[c]["afin"].astype(np.float64)   # (128, SW+2) mirrored
        gfin = res.results[c]["gfin"].astype(np.float64)   # (128, SW) mirrored
        hist = res.results[c]["hist"].astype(np.float64)
        acc = np.log(np.maximum(hist[:, :NRESC], 1e-300)).sum(axis=1)
        afs = afin[:, 0:SW][:, ::-1]             # un-mirror -> natural order
        gfs = gfin[:, :][:, ::-1]
        af = afs[0:64, 0:S]                      # alpha_{T/2-1}[s]
        gm = gfs[64:128, 0:S][:, ::-1]           # W_{T/2-1}[s], u -> s
        P = (af * gm).sum(axis=1)
        lnP = np.log(np.maximum(P, 1e-300))
        losses[c * BN:(c + 1) * BN] = -(
            lnP + acc[:64] + acc[64:128] - 2 * setlog - T * lng)
    return np.float32(losses.mean())
